# revision 4
# baseline (speedup 1.0000x reference)
"""Trainium2 Bass kernel for nn_MultiHeadAttention_91311004713343.

Full-input contract: kernel(**inputs) takes the unsharded numpy inputs and
returns the full (B, T, C) float32 output.

Sharding (8 cores): core c handles batch b = c//4 and head group g = c%4
(4 of the 16 heads). Per core the device program computes, in fp16 with
fp32 PSUM accumulation:
  1. q/k/v projections for its 4 heads (contraction over C), with RoPE
     (+ qkv bias) fused into the PSUM eviction. q/k are produced in
     head-transposed [D, T] layout, v in natural [T, D] layout.
  2. Attention per head: S^T tiles = kT x qT on PE, exp (with 1/sqrt(D)
     scale folded in) on ScalarE, y^T accumulation and softmax row-sums
     on PE, normalization fused into the y eviction.
  3. Output projection partial (its 512 of 2048 contraction channels),
     producing out^T [C, T] fp32; b_proj is added on the g==0 core only.
Host side only reshapes/transposes/casts inputs and sums/transposes the
8 partial outputs (the TP all-reduce + gather).
"""

import sys

for _p in ("/opt/trn_rl_repo",):
    if _p not in sys.path:
        sys.path.insert(0, _p)

import numpy as np

import concourse.bass as bass
import concourse.mybir as mybir
import concourse.tile as tile
from concourse import bacc
from concourse.bass_utils import run_bass_kernel_spmd

B, T, C = 2, 2048, 2048
H, D = 16, 128
NCORES = 8
HPC = 4           # heads per core
CL = HPC * D      # local contraction channels (512)
TS = 512          # t-slice (free dim per matmul)
NTS = T // TS     # 4
NKC = T // 128    # 16 key chunks
NCC = C // 128    # 16 contraction chunks
F16 = mybir.dt.float16
F32 = mybir.dt.float32
SCALE = 1.0 / float(np.sqrt(D))

_CACHE = {}


def build_nc():
    nc = bacc.Bacc("TRN2", target_bir_lowering=False, debug=False)

    xT = nc.dram_tensor("xT", [C, T], F16, kind="ExternalInput")
    wq = nc.dram_tensor("wq", [C, CL], F16, kind="ExternalInput")
    wk = nc.dram_tensor("wk", [C, CL], F16, kind="ExternalInput")
    wv = nc.dram_tensor("wv", [C, CL], F16, kind="ExternalInput")
    wp = nc.dram_tensor("wp", [CL, C], F16, kind="ExternalInput")
    cosT = nc.dram_tensor("cosT", [D, T], F16, kind="ExternalInput")
    sinT = nc.dram_tensor("sinT", [D, T], F16, kind="ExternalInput")
    bq = nc.dram_tensor("bq", [D, HPC], F32, kind="ExternalInput")
    bqr = nc.dram_tensor("bqr", [D, HPC], F32, kind="ExternalInput")
    bk = nc.dram_tensor("bk", [D, HPC], F32, kind="ExternalInput")
    bkr = nc.dram_tensor("bkr", [D, HPC], F32, kind="ExternalInput")
    bv = nc.dram_tensor("bv", [1, HPC * D], F16, kind="ExternalInput")
    bp = nc.dram_tensor("bp", [128, NCC], F32, kind="ExternalInput")
    outT = nc.dram_tensor("outT", [C, T], F32, kind="ExternalOutput")

    ADD = mybir.AluOpType.add
    MULT = mybir.AluOpType.mult
    AF = mybir.ActivationFunctionType

    from contextlib import ExitStack

    with tile.TileContext(nc) as tc, ExitStack() as ctx:
        const = ctx.enter_context(tc.tile_pool(name="const", bufs=1))
        persist = ctx.enter_context(tc.tile_pool(name="persist", bufs=1))

        # constants / weights resident in SBUF
        cos_sb = const.tile([D, T], F16, tag="cos")
        nc.sync.dma_start(cos_sb[:], cosT[:])
        sin_sb = const.tile([D, T], F16, tag="sin")
        nc.sync.dma_start(sin_sb[:], sinT[:])
        bq_sb = const.tile([D, HPC], F32, tag="bq")
        nc.sync.dma_start(bq_sb[:], bq[:])
        bqr_sb = const.tile([D, HPC], F32, tag="bqr")
        nc.sync.dma_start(bqr_sb[:], bqr[:])
        bk_sb = const.tile([D, HPC], F32, tag="bk")
        nc.sync.dma_start(bk_sb[:], bk[:])
        bkr_sb = const.tile([D, HPC], F32, tag="bkr")
        nc.sync.dma_start(bkr_sb[:], bkr[:])
        bv_sb = const.tile([1, HPC * D], F16, tag="bv")
        nc.sync.dma_start(bv_sb[:], bv[:])
        bp_sb = const.tile([128, NCC], F32, tag="bp")
        nc.sync.dma_start(bp_sb[:], bp[:])
        ones_sb = const.tile([128, 1], F16, tag="ones")
        nc.vector.memset(ones_sb[:], 1.0)

        wq_sb, wk_sb, wv_sb = [], [], []
        for cc in range(NCC):
            for (lst, dram, nm) in ((wq_sb, wq, "wq"), (wk_sb, wk, "wk"),
                                    (wv_sb, wv, "wv")):
                t_ = const.tile([128, CL], F16, tag=f"{nm}{cc}", name=f"{nm}{cc}")
                nc.sync.dma_start(t_[:], dram[cc * 128:(cc + 1) * 128, :])
                lst.append(t_)
        wp_sb = []
        for hc in range(HPC):
            t_ = const.tile([128, C], F16, tag=f"wp{hc}", name=f"wp{hc}")
            nc.sync.dma_start(t_[:], wp[hc * 128:(hc + 1) * 128, :])
            wp_sb.append(t_)

        # persistent activations
        qT_sb = [persist.tile([D, T], F16, tag=f"qT{h}", name=f"qT{h}") for h in range(HPC)]
        kT_sb = [persist.tile([D, T], F16, tag=f"kT{h}", name=f"kT{h}") for h in range(HPC)]
        v_sb = [persist.tile([128, CL], F16, tag=f"v{kc}", name=f"v{kc}") for kc in range(NKC)]
        yT_sb = [persist.tile([D, T], F16, tag=f"yT{h}", name=f"yT{h}") for h in range(HPC)]

        # ---------------- phase 1: qkv projection + rope ----------------
        with tc.tile_pool(name="xb", bufs=2) as xpool, \
             tc.tile_pool(name="p1ps", bufs=2, space="PSUM") as p1ps, \
             tc.tile_pool(name="rope", bufs=3) as rpool:
            for ts in range(NTS):
                tsl = bass.ts(ts, TS)
                xb = []
                for cc in range(NCC):
                    t_ = xpool.tile([128, TS], F16, tag=f"xb{cc}", name=f"xb{cc}")
                    nc.sync.dma_start(t_[:], xT[cc * 128:(cc + 1) * 128, tsl])
                    xb.append(t_)
                for j in range(HPC):
                    jsl = bass.ts(j, 128)
                    for (w_sb, b_sb, br_sb, dest) in (
                            (wq_sb, bq_sb, bqr_sb, qT_sb),
                            (wk_sb, bk_sb, bkr_sb, kT_sb)):
                        ps = p1ps.tile([128, TS], F32, tag="ps")
                        for cc in range(NCC):
                            nc.tensor.matmul(ps[:], w_sb[cc][:, jsl], xb[cc][:],
                                             start=(cc == 0), stop=(cc == NCC - 1))
                        # rope: dest = (ps+b)*cos + (ps_swap+b_rot)*sin_signed
                        tco = rpool.tile([128, TS], F16, tag="tcos")
                        nc.vector.scalar_tensor_tensor(
                            tco[:], ps[:], b_sb[:, j:j + 1], cos_sb[:, tsl],
                            op0=ADD, op1=MULT)
                        tsi = rpool.tile([128, TS], F16, tag="tsin")
                        nc.vector.scalar_tensor_tensor(
                            tsi[0:64, :], ps[64:128, :], br_sb[0:64, j:j + 1],
                            sin_sb[0:64, tsl], op0=ADD, op1=MULT)
                        nc.vector.scalar_tensor_tensor(
                            tsi[64:128, :], ps[0:64, :], br_sb[64:128, j:j + 1],
                            sin_sb[64:128, tsl], op0=ADD, op1=MULT)
                        nc.vector.tensor_add(dest[j][:, tsl], tco[:], tsi[:])
                for j in range(HPC):
                    jsl = bass.ts(j, 128)
                    ps = p1ps.tile([128, TS], F32, tag="ps")
                    for cc in range(NCC):
                        nc.tensor.matmul(ps[:], xb[cc][:, jsl], wv_sb[cc][:],
                                         start=(cc == 0), stop=(cc == NCC - 1))
                    nc.scalar.copy(v_sb[ts * HPC + j][:], ps[:])

        # ---------------- phase 2: attention ----------------
        with tc.tile_pool(name="sps", bufs=2, space="PSUM") as sps, \
             tc.tile_pool(name="yps", bufs=2, space="PSUM") as yps, \
             tc.tile_pool(name="rps", bufs=2, space="PSUM") as rps, \
             tc.tile_pool(name="pp", bufs=4) as ppool, \
             tc.tile_pool(name="sm", bufs=2) as smpool:
            for h in range(HPC):
                hsl = bass.ts(h, 128)
                for qs in range(NTS):
                    qsl = bass.ts(qs, TS)
                    ps_y = yps.tile([128, TS], F32, tag="y")
                    ps_r = rps.tile([1, TS], F32, tag="r")
                    for kc in range(NKC):
                        ps_s = sps.tile([128, TS], F32, tag="s")
                        nc.tensor.matmul(ps_s[:], kT_sb[h][:, bass.ts(kc, 128)],
                                         qT_sb[h][:, qsl], start=True, stop=True)
                        p_sb = ppool.tile([128, TS], F16, tag="p")
                        nc.scalar.activation(p_sb[:], ps_s[:], AF.Exp, scale=SCALE)
                        nc.tensor.matmul(ps_y[:], v_sb[kc][:, hsl], p_sb[:],
                                         start=(kc == 0), stop=False)
                        nc.tensor.matmul(ps_r[:], ones_sb[:], p_sb[:],
                                         start=(kc == 0), stop=(kc == NKC - 1))
                    r_sb = smpool.tile([1, TS], F32, tag="rsb")
                    nc.scalar.copy(r_sb[:], ps_r[:])
                    r16 = smpool.tile([1, TS], F16, tag="r16")
                    nc.vector.tensor_copy(r16[:], r_sb[:])
                    # v-bias term: y += b_v outer r  (=> +b_v after normalize)
                    nc.tensor.matmul(ps_y[:], bv_sb[:, hsl], r16[:],
                                     start=False, stop=True)
                    recip = smpool.tile([1, TS], F32, tag="recip")
                    nc.vector.reciprocal(recip[:], r_sb[:])
                    recip_b = smpool.tile([128, TS], F32, tag="recipb")
                    nc.gpsimd.partition_broadcast(recip_b[:], recip[:])
                    nc.vector.tensor_mul(yT_sb[h][:, qsl], ps_y[:], recip_b[:])

        # ---------------- phase 3: output projection ----------------
        with tc.tile_pool(name="ops", bufs=2, space="PSUM") as ops, \
             tc.tile_pool(name="osb", bufs=3) as opool:
            for oc in range(NCC):
                ocl = bass.ts(oc, 128)
                for ts in range(NTS):
                    tsl = bass.ts(ts, TS)
                    ps_o = ops.tile([128, TS], F32, tag="o")
                    for hc in range(HPC):
                        nc.tensor.matmul(ps_o[:], wp_sb[hc][:, ocl],
                                         yT_sb[hc][:, tsl],
                                         start=(hc == 0), stop=(hc == HPC - 1))
                    o_sb = opool.tile([128, TS], F32, tag="osb")
                    nc.scalar.activation(o_sb[:], ps_o[:], AF.Identity,
                                         bias=bp_sb[:, oc:oc + 1])
                    nc.sync.dma_start(outT[oc * 128:(oc + 1) * 128, tsl], o_sb[:])

    nc.compile()
    return nc


def shard_inputs(x, cos, sin, W_qkv, b_qkv, W_proj, b_proj):
    f16 = np.float16
    WqkvT = np.ascontiguousarray(W_qkv.T)  # [C, 3C]
    WpT = W_proj.T                         # [C, C]
    cosT = np.ascontiguousarray(cos.T).astype(f16)
    sinT_s = np.ascontiguousarray(sin.T).copy()
    sinT_s[0:D // 2, :] *= -1.0
    sinT_s = sinT_s.astype(f16)
    xT = [np.ascontiguousarray(x[b].T).astype(f16) for b in range(B)]

    def btile(vec):  # [CL] -> [128, HPC] fp32 (col j = head j's slice)
        return np.ascontiguousarray(vec.reshape(HPC, D).T).astype(np.float32)

    in_maps = []
    for c in range(NCORES):
        b, g = c // HPC, c % HPC
        sl = slice(g * CL, (g + 1) * CL)
        bq_v = b_qkv[0 * C:1 * C][sl]
        bk_v = b_qkv[1 * C:2 * C][sl]
        bv_v = b_qkv[2 * C:3 * C][sl]
        in_maps.append({
            "xT": xT[b],
            "wq": np.ascontiguousarray(WqkvT[:, 0 * C:1 * C][:, sl]).astype(f16),
            "wk": np.ascontiguousarray(WqkvT[:, 1 * C:2 * C][:, sl]).astype(f16),
            "wv": np.ascontiguousarray(WqkvT[:, 2 * C:3 * C][:, sl]).astype(f16),
            "wp": np.ascontiguousarray(WpT[sl, :]).astype(f16),
            "cosT": cosT,
            "sinT": sinT_s,
            "bq": btile(bq_v),
            "bqr": np.roll(btile(bq_v), D // 2, axis=0),
            "bk": btile(bk_v),
            "bkr": np.roll(btile(bk_v), D // 2, axis=0),
            "bv": np.ascontiguousarray(bv_v.reshape(1, HPC * D)).astype(f16),
            "bp": (np.ascontiguousarray(b_proj.reshape(NCC, 128).T)
                   .astype(np.float32) if g == 0
                   else np.zeros((128, NCC), np.float32)),
        })
    return in_maps


def unshard(results):
    out = np.empty((B, T, C), np.float32)
    for b in range(B):
        acc = results[b * HPC]["outT"].astype(np.float32)
        for g in range(1, HPC):
            acc = acc + results[b * HPC + g]["outT"]
        out[b] = acc.T
    return out


def kernel(**inputs):
    if "nc" not in _CACHE:
        _CACHE["nc"] = build_nc()
    nc = _CACHE["nc"]
    in_maps = shard_inputs(**{k: np.asarray(v) for k, v in inputs.items()})
    res = run_bass_kernel_spmd(nc, in_maps, core_ids=list(range(NCORES)))
    return unshard(res.results)


if __name__ == "__main__":
    import reference

    inputs = {k: np.asarray(v) for k, v in reference.setup_inputs().items()}
    out = kernel(**inputs)
    import jax
    expected = np.asarray(reference.reference(**inputs))
    rel = np.linalg.norm(out - expected) / np.linalg.norm(expected)
    print("rel l2:", rel)
